# revision 37
# baseline (speedup 1.0000x reference)
"""LIF spiking-neuron kernel for Trainium2 (Bass/Tile), 8-core SPMD.

Problem: x [B=32, T=8, C=128, H=32, W=32] f32.  Per (b,c,h,w) neuron,
sequential over T:
    mem = mem*TAU + x_t;  spike = (mem - 1 > 0);  mem = 0 if spike
TAU = 0.5, THRESH = 1.0.

Sharding: batch dim B=32 split 4-per-core across 8 NeuronCores; the
recurrence is along T only, so there is no communication.

Per-core algorithm (bit-exact vs the fp32 reference):
  TAU = 0.5 is a power of two, so rescale the state M_t = 2^t * m_t.
  The decay becomes a pure add:  M_t = M_{t-1} + 2^t * x_t  (the 2^t
  prescale of x is exact in fp32, and power-of-2 scaling commutes with
  fp rounding, so every M_t is bit-exactly 2^t * m_t).
  spike_t = (M_t > 2^t)  <=>  (m_t > 1)  <=>  reference's (m_t - 1 > 0).

  The whole state update (previous step's reset + prescale + accumulate)
  is ONE fused custom-DVE op per step:
      M_t = select(M_{t-1} > 2^(t-1), 0, M_{t-1}) + x_t * 2^t
  so the spike computation is only an output tap, off the recurrence
  critical path.  Spikes are written to DRAM as uint8 (4x less output
  DMA traffic than f32) and upcast to f32 on the host.

Engine split per (g,t) tile of [C=128 partitions, PAIR*H*W=2048]:
  DVE:  12 LIF_STEP + 2 fused LIF_SPIKE (t=7 emits the u8 spike
        directly -- no final membrane, no separate compare, no
        cross-engine hop on the critical tail)
  ACT:  all t<7 spikes: Sign(M - 2^t) -> u8 in ONE op (the saturating
        u8 cast maps Sign's -1 to 0, verified on HW)
  SP :  the single in-order DMA queue: all inputs, then all outputs --
        output traffic never delays the input stream that paces the
        chain, and the output drain hides the compute tail
  GPS:  nothing -- its software ops are ~15x slower than DVE and it
        shares SBUF ports with DVE (running anything there starves the
        recurrence chain)

DMA layout: host pre-transposes x to [G, T, C, PAIR*H*W] and decodes
the same layout on output, so every DMA is one contiguous line per
partition (8 KB in / 2 KB out, 128 descriptors) instead of two half
lines -- fewer, larger descriptors keep the 16 DMA engines balanced
(the [b,t,c,h,w] layout left engine 15 a ~6 us straggler).
"""

import re

import numpy as np

from concourse import bacc, bass, mybir, tile
from concourse import dve_ops
from concourse.alu_op_type import AluOpType
from concourse.bass_utils import run_bass_kernel_spmd
from concourse.dve_spec import Spec, Src0, Src1, C0, C1, Zero, select

# Full-problem shape (hardcoded per harness contract).
B, T, C, H, W = 32, 8, 128, 32, 32
N_CORES = 8
B_LOC = B // N_CORES          # 4 batches per core
F = H * W                     # 1024 free elements per (b, t, c)
FP32 = mybir.dt.float32
U8 = mybir.dt.uint8
FP8 = mybir.dt.float8e4

PAIR = 2                      # batches fused per tile
G = B_LOC // PAIR             # chain groups per core
FW = PAIR * F                 # 2048 free elements per tile
BANK = 512                    # PSUM bank: one matmul output's max f32 cols
N_LO = 4                      # t=0..3 trit-packed into plane_lo
BIAS_LO = 85.0                # (4^4 - 1) / 3

_NC_CACHE = {}


def _register_lif_op():
    """Register the fused LIF state-update custom-DVE op (idempotent).

    out = select(in1 > s1, 0, in1) + in0 * s0
        = (previous step's hard reset) + (2^t-prescaled input)
    """
    name = "LIF_STEP_ANT"
    if name in dve_ops._SUB_OPCODE_FOR_NAME:
        return next(op for op in dve_ops.OPS if op.name == name)
    body = select(Src1 > C1, Zero, Src1) + Src0 * C0
    op = dve_ops.DveOp(
        name,
        Spec(
            body=body,
            reference=lambda in0, in1, s0, s1, imm2: (
                np.where(in1 > s1, np.float32(0.0), in1) + in0 * s0
            ).astype(np.float32),
        ),
        subdim=False,
        uops_sha={},
    )
    dve_ops.OPS.append(op)
    dve_ops.CUSTOM_DVE_SPECS[name] = op.spec
    dve_ops._SUB_OPCODE_FOR_NAME[name] = (
        dve_ops._CUSTOM_DVE_ROW_BASE + len(dve_ops.OPS) - 1
    )
    # Pin uops_sha to whatever lower() produces in this process.
    for ver in ("v3", "v4"):
        try:
            op.compile(ver)
        except ValueError as e:
            m = re.search(r'"%s"\]="([0-9a-f]{16})"' % ver, str(e))
            if not m:
                raise
            op.uops_sha[ver] = m.group(1)
            dve_ops._COMPILE_CACHE.pop((name, ver), None)
            op.compile(ver)
    return op


LIF_STEP = _register_lif_op()


def _register_lif_spike_op():
    """Fused final step: spike = (select(in1 > s1, 0, in1) + in0*s0) > s0.

    Used for t = T-1 where the new membrane is never needed again: one
    DVE op produces the u8 spike directly, shortening the critical tail
    (no separate compare, no cross-engine hop).
    """
    name = "LIF_SPIKE_ANT"
    if name in dve_ops._SUB_OPCODE_FOR_NAME:
        return next(op for op in dve_ops.OPS if op.name == name)
    body = (select(Src1 > C1, Zero, Src1) + Src0 * C0) > C0
    op = dve_ops.DveOp(
        name,
        Spec(
            body=body,
            reference=lambda in0, in1, s0, s1, imm2: (
                (np.where(in1 > s1, np.float32(0.0), in1) + in0 * s0) > s0
            ).astype(np.float32),
        ),
        subdim=False,
        uops_sha={},
    )
    dve_ops.OPS.append(op)
    dve_ops.CUSTOM_DVE_SPECS[name] = op.spec
    dve_ops._SUB_OPCODE_FOR_NAME[name] = (
        dve_ops._CUSTOM_DVE_ROW_BASE + len(dve_ops.OPS) - 1
    )
    for ver in ("v3", "v4"):
        try:
            op.compile(ver)
        except ValueError as e:
            m = re.search(r'"%s"\]="([0-9a-f]{16})"' % ver, str(e))
            if not m:
                raise
            op.uops_sha[ver] = m.group(1)
            dve_ops._COMPILE_CACHE.pop((name, ver), None)
            op.compile(ver)
    return op


LIF_SPIKE = _register_lif_spike_op()


def _emit(tc, x_d, w_d, o_lo, o_raw):
    nc = tc.nc

    # DRAM is pre-transposed host-side to [G, T, C, PAIR*F], so every
    # DMA moves one contiguous line per partition (8 KB in / 2 KB out,
    # 128 descriptors) instead of two half lines (256 descriptors) --
    # fewer, larger descriptors keep the 16 DMA engines balanced.

    with (
        tc.tile_pool(name="xp", bufs=12) as xp,
        tc.tile_pool(name="sp", bufs=10) as sp,
        tc.tile_pool(name="sg", bufs=4) as sgp,
        tc.tile_pool(name="mp", bufs=6) as mp,
        tc.tile_pool(name="bp", bufs=1) as bp,
        tc.tile_pool(name="ps", bufs=G, space=bass.MemorySpace.PSUM) as ps,
    ):
        # per-t [128,1] bias columns holding -2^t for the ACT Sign compare.
        # NOTE: keep GpSimd completely idle — its software ops are ~15x
        # slower than DVE and it shares SBUF ports with DVE (running
        # anything there starves the recurrence chain).
        biases = []
        for t in range(T):
            bt = bp.tile([C, 1], FP32, name=f"bias{t}")
            nc.vector.memset(bt, -float(2.0**t))
            biases.append(bt)
        bias_lo = bp.tile([C, 1], FP32, name="bias_lo")
        nc.vector.memset(bias_lo, BIAS_LO)

        # 4^t diagonal fp8 pack weights: one small contiguous DMA on the
        # ACT queue so the SP input stream starts clean
        wall = bp.tile([C, N_LO * C], FP8, name="wall")
        nc.scalar.dma_start(out=wall, in_=w_d)
        wts = [wall[:, j * C : (j + 1) * C] for j in range(N_LO)]

        # --- all input DMAs issued up front, t-major, SP/ACT split by chain.
        # t=0 lands directly in the chain's first membrane tile (M_0 = x_0).
        # Everything on the single in-order SP queue: inputs first, then
        # outputs.  Outputs therefore drain only after the input stream
        # finishes -- deliberate: input arrival (which paces the LIF
        # chain) is never slowed by output traffic, and the ~11us output
        # drain swallows the compute tail.  Spike tiles get dedicated
        # buffers (bufs = T*G) so the late drain frees nothing anyone
        # waits on.
        # first tiles split into quarters: descriptor generation for a
        # quarter is ~4x faster, so the first HBM transfer (which gates
        # the whole stream's start) begins ~1us earlier
        QF = FW // 4
        ms = {}
        for g in range(G):
            m0 = mp.tile([C, FW], FP32, name="mt")
            for q in range(4):
                sl = slice(q * QF, (q + 1) * QF)
                nc.sync.dma_start(out=m0[:, sl], in_=x_d[g, 0][:, sl])
            ms[g] = m0
        HF = FW // 2
        xs = {}
        for t in range(1, T):
            for g in range(G):
                xt = xp.tile([C, FW], FP32)
                if t < T - 1:
                    nc.sync.dma_start(out=xt, in_=x_d[g, t])
                else:
                    # final step split into halves: the first half's spike
                    # and output overlap the second half's transfer
                    for h in range(2):
                        sl = slice(h * HF, (h + 1) * HF)
                        nc.sync.dma_start(out=xt[:, sl], in_=x_d[g, t][:, sl])
                xs[(t, g)] = xt

        # --- recurrence (DVE) + spike taps (ACT) + u8 output DMAs.
        # All t<7 spikes on ACT (one Sign op each) so DVE carries ONLY the
        # recurrence -- under clock throttling 14 LIFs + 8 compares would
        # overrun the input stream and pace the tail.  t=7 uses the fused
        # LIF_SPIKE op: one DVE op emits the u8 spike directly (no final
        # membrane, no separate compare, no cross-engine hop on the tail).
        accs = [ps.tile([C, FW], FP32, name="acc") for _ in range(G)]

        for t in range(T):
            th = float(2.0**t)
            for g in range(G):
                if 0 < t < T - 1:
                    m_new = mp.tile([C, FW], FP32, name="mt")
                    nc.vector._custom_dve(
                        LIF_STEP,
                        out=m_new,
                        in0=xs[(t, g)],
                        in1=ms[g],
                        s0=th,
                        s1=th / 2.0,
                    )
                    ms[g] = m_new
                if t < N_LO:
                    # trit sigma_t = Sign(M - 2^t) in fp8 {-1,0,+1}, packed
                    # by the idle PE into PSUM with 4^t diagonal weights
                    sg = sgp.tile([C, FW], FP8, name="sgt")
                    nc.scalar.activation(
                        sg, ms[g], mybir.ActivationFunctionType.Sign, bias=biases[t]
                    )
                    for j in range(FW // BANK):
                        sl = slice(j * BANK, (j + 1) * BANK)
                        nc.tensor.matmul(
                            accs[g][:, sl], wts[t], sg[:, sl],
                            start=(t == 0), stop=(t == N_LO - 1),
                        )
                    if t == N_LO - 1:
                        pk = sp.tile([C, FW], U8, name="pk")
                        nc.scalar.activation(
                            pk, accs[g],
                            mybir.ActivationFunctionType.Identity, bias=bias_lo,
                        )
                        nc.sync.dma_start(out=o_lo[g], in_=pk)
                elif t < T - 1:
                    # raw u8 spike: Sign's u8 cast saturates -1 -> 0
                    s = sp.tile([C, FW], U8, name="pk")
                    nc.scalar.activation(
                        s, ms[g], mybir.ActivationFunctionType.Sign, bias=biases[t]
                    )
                    nc.sync.dma_start(out=o_raw[t - N_LO][g], in_=s)
                else:
                    s = sp.tile([C, FW], U8, name="pk")
                    for h in range(2):
                        sl = slice(h * HF, (h + 1) * HF)
                        nc.vector._custom_dve(
                            LIF_SPIKE,
                            out=s[:, sl],
                            in0=xs[(t, g)][:, sl],
                            in1=ms[g][:, sl],
                            s0=th,
                            s1=th / 2.0,
                        )
                        nc.sync.dma_start(
                            out=o_raw[t - N_LO][g][:, sl], in_=s[:, sl]
                        )


def build_nc():
    """Build + compile the per-core Bass program (cached)."""
    if "nc" in _NC_CACHE:
        return _NC_CACHE["nc"]
    nc = bacc.Bacc(
        "TRN2",
        target_bir_lowering=False,
        debug=False,
        enable_asserts=False,
        num_devices=N_CORES,
    )
    x_d = nc.dram_tensor("x", [G, T, C, FW], FP32, kind="ExternalInput").ap()
    w_d = nc.dram_tensor("w", [C, N_LO * C], FP8, kind="ExternalInput").ap()
    o_lo = nc.dram_tensor("out_lo", [G, C, FW], U8, kind="ExternalOutput").ap()
    o_raw = [
        nc.dram_tensor(f"out_t{t}", [G, C, FW], U8, kind="ExternalOutput").ap()
        for t in range(N_LO, T)
    ]
    with tile.TileContext(nc) as tc:
        _emit(tc, x_d, w_d, o_lo, o_raw)
    nc.compile()
    _NC_CACHE["nc"] = nc
    return nc


def make_in_maps(x: np.ndarray) -> list[dict[str, np.ndarray]]:
    assert x.shape == (B, T, C, H, W) and x.dtype == np.float32, (x.shape, x.dtype)
    np_fp8 = mybir.dt.np(FP8)
    w = np.zeros((C, N_LO, C), dtype=np_fp8)
    for j in range(N_LO):
        np.fill_diagonal(w[:, j, :], np_fp8(4.0**j))
    w = np.ascontiguousarray(w.reshape(C, N_LO * C))
    maps = []
    for i in range(N_CORES):
        xc = x[i * B_LOC : (i + 1) * B_LOC].reshape(G, PAIR, T, C, F)
        xc = np.ascontiguousarray(xc.transpose(0, 2, 3, 1, 4)).reshape(G, T, C, FW)
        maps.append({"x": xc, "w": w})
    return maps


def kernel(x: np.ndarray) -> np.ndarray:
    x = np.asarray(x, dtype=np.float32)
    nc = build_nc()
    res = run_bass_kernel_spmd(nc, make_in_maps(x), list(range(N_CORES)))

    def plane(name):
        # [G, C, PAIR*F] u8 per core -> [B, C, H, W]
        parts = []
        for r in res.results:
            oc = r[name].reshape(G, C, PAIR, F).transpose(0, 2, 1, 3)
            parts.append(oc.reshape(B_LOC, C, H, W))
        return np.concatenate(parts, axis=0)

    out = np.empty((B, T, C, H, W), dtype=np.float32)
    # balanced-quaternary decode of plane_lo: highest digit first
    d = plane("out_lo").astype(np.int16) - int(BIAS_LO)
    for t in range(N_LO - 1, -1, -1):
        wt = 4**t
        mt = (wt - 1) // 3  # max |sum of remaining lower digits|
        st = (d > mt).astype(np.int16) - (d < -mt).astype(np.int16)
        d -= st * wt
        out[:, t] = st == 1
    for t in range(N_LO, T):
        out[:, t] = plane(f"out_t{t}")
    return out
